# revision 10
# baseline (speedup 1.0000x reference)
"""Trainium2 Bass kernel for the BoW language model head problem.

Model (per reference):
    emb = wte[x] + wpe            (B,T,C)
    h   = emb + cumsum(emb)/[1..T]
    h   = h + tanh(h@w_fc+b_fc)@w_proj + b_proj
    out = h @ w_head + b_head     (B,T,V)

Shapes: B=4, T=2048, V=50257, C=512, H=2048.

Sharding (8 cores): core i computes batch i//2 and vocab half i%2.
Pre-head compute is split across the pair by tokens (each core does its
own 1024-token half); a bf16 AllReduce per 512-token group plus a
subtract reconstructs the peer half, overlapped with the own-half head
matmuls.  Data is bf16 end-to-end with fp32 PSUM accumulation; the
embedding gather runs on the host, and the causal-BoW cumsum folds 1/t
into a per-block matrix M1 = I + triu*recip so its matmul lands h
directly in C-major layout (no PE transposes).  The head streams w_head
tiles (moving, N=512) against stationary h blocks; output is written
bf16 and up-converted on the host.
"""

from contextlib import ExitStack

import numpy as np
import ml_dtypes

import concourse.bacc as bacc
import concourse.bass as bass
import concourse.mybir as mybir
import concourse.tile as tile
from concourse.bass_utils import run_bass_kernel_spmd

P = 128
B, T, V, C, H = 4, 2048, 50257, 512, 2048
NBLK = T // P          # 16 token blocks
NLOC = NBLK // 2       # 8 local token blocks per core (pair-split pre-head)
TLOC = NLOC * P        # 1024 local tokens
NCC = C // P           # 4 C chunks
NHC = H // P           # 16 H chunks
TG = 512               # token group width (MLP moving dim)
VT = 512               # vocab tile width
VSPLIT = (V + 1) // 2  # 25129: half0 = [:VSPLIT], half1 = [VSPLIT:]
VHALF_PAD = 25216      # VSPLIT padded to a multiple of 128 (49*512 + 128)
# vocab tile widths and groups (~2048 elements per group -> 4 PSUM banks
# per j-block, ping-pong across j)
_TILES = [(i * VT, VT) for i in range(VHALF_PAD // VT)]
if VHALF_PAD % VT:
    _TILES.append((VHALF_PAD - VHALF_PAD % VT, VHALF_PAD % VT))
VGROUPS = [_TILES[i:i + 4] for i in range(0, len(_TILES), 4)]

F32 = mybir.dt.float32
BF16 = mybir.dt.bfloat16

BF = ml_dtypes.bfloat16


def _build_nc():
    nc = bacc.Bacc(None, target_bir_lowering=False, debug=True,
                   num_swdge_queues=4, num_devices=8)

    emb = nc.dram_tensor("emb", [P, NBLK, C], BF16, kind="ExternalInput")
    w_fc = nc.dram_tensor("w_fc", [P, NCC, H], BF16, kind="ExternalInput")
    w_proj = nc.dram_tensor("w_proj", [P, NHC, C], BF16, kind="ExternalInput")
    w_head = nc.dram_tensor("w_head", [C, VHALF_PAD], BF16,
                            kind="ExternalInput")
    b_fc2d = nc.dram_tensor("b_fc2d", [P, NHC], F32, kind="ExternalInput")
    b_proj2d = nc.dram_tensor("b_proj2d", [P, NCC], F32, kind="ExternalInput")
    m1 = nc.dram_tensor("m1", [P, NLOC, P], BF16, kind="ExternalInput")
    rrow = nc.dram_tensor("rrow", [1, TLOC], BF16, kind="ExternalInput")
    mask = nc.dram_tensor("mask", [1, 1], F32, kind="ExternalInput")
    ones_col = nc.dram_tensor("ones_col", [P, 1], BF16, kind="ExternalInput")
    out = nc.dram_tensor("out", [T, VHALF_PAD], BF16, kind="ExternalOutput")

    with tile.TileContext(nc) as tc:
        with tc.tile_pool(name="consts", bufs=1) as consts, \
             tc.tile_pool(name="hfp", bufs=1) as hfp, \
             tc.tile_pool(name="whp", bufs=16) as whp, \
             tc.tile_pool(name="ccdr", bufs=1, space="DRAM") as ccdr:
            ones_sb = consts.tile([P, 1], BF16, tag="ones")
            nc.sync.dma_start(out=ones_sb[:], in_=ones_col[:])

            # hF holds post-MLP h (C-major, bf16): own tokens 0:1024,
            # peer tokens 1024:2048 (reconstructed after the AllReduce).
            hF = hfp.tile([P, NCC, T], BF16, tag="hF")
            cc_in = [ccdr.tile([P, NCC, TG], BF16, tag=f"cci{g}",
                               name=f"cc_in{g}")
                     for g in range(2)]
            cc_red = [ccdr.tile([P, NCC, TG], BF16, tag=f"ccr{g}",
                                name=f"cc_red{g}")
                      for g in range(2)]

            wh_view = w_head.rearrange("(c p) v -> p c v", p=P)

            def load_group(group):
                whs = []
                for v0, w in group:
                    wh = whp.tile([P, NCC, VT], BF16, tag="wh")
                    nc.sync.dma_start(out=wh[:, :, :w],
                                        in_=wh_view[:, :, v0:v0 + w])
                    whs.append(wh)
                return whs

            stack_bc = ExitStack()
            htp = stack_bc.enter_context(tc.tile_pool(name="htp", bufs=1))
            hTpre = htp.tile([P, NCC, TLOC], BF16, tag="hTpre")

            # ---------------- Phase B: causal BoW ----------
            with tc.tile_pool(name="ebuf", bufs=1) as ebuf, \
                 tc.tile_pool(name="ssp", bufs=3) as ssp, \
                 tc.tile_pool(name="pss", bufs=2, space="PSUM") as pss, \
                 tc.tile_pool(name="psh", bufs=2, space="PSUM") as psh:
                E = ebuf.tile([P, NBLK, C], BF16, tag="E")
                # spread the critical early loads across the three DMA
                # paths: peer-E on sync, own-E on scalar, consts on gpsimd
                nc.sync.dma_start(out=E[:, NLOC:, :], in_=emb[:, NLOC:, :])
                nc.scalar.dma_start(out=E[:, :NLOC, :], in_=emb[:, :NLOC, :])
                m1_sb = consts.tile([P, NLOC, P], BF16, tag="m1")
                nc.gpsimd.dma_start(out=m1_sb[:], in_=m1[:])
                rrow_sb = consts.tile([1, TLOC], BF16, tag="rrow")
                nc.gpsimd.dma_start(out=rrow_sb[:], in_=rrow[:])
                mask_sb = consts.tile([1, 1], F32, tag="mask")
                nc.gpsimd.dma_start(out=mask_sb[:], in_=mask[:])
                bfc_sb = consts.tile([P, NHC], F32, tag="bfc")
                nc.gpsimd.dma_start(out=bfc_sb[:], in_=b_fc2d[:])
                bproj_sb = consts.tile([P, NCC], F32, tag="bproj")
                nc.gpsimd.dma_start(out=bproj_sb[:], in_=b_proj2d[:])

                # prefix base: colsum over the other half's blocks, masked
                # (mask=1 iff this core owns the second global half).
                ps_base = pss.tile([1, C], F32, tag="cs")
                for j in range(NLOC, NBLK):
                    nc.tensor.matmul(ps_base[:], lhsT=ones_sb[:],
                                     rhs=E[:, j, :],
                                     start=(j == NLOC), stop=(j == NBLK - 1))
                s_f32 = ssp.tile([1, C], F32, tag="sf")
                nc.scalar.activation(s_f32[:], ps_base[:],
                                     mybir.ActivationFunctionType.Copy,
                                     scale=mask_sb[:, :1])
                s_bf = ssp.tile([1, C], BF16, tag="sb")
                nc.vector.tensor_copy(s_bf[:], s_f32[:])

                for j in range(NLOC):
                    ph = psh.tile([P, NCC, P], F32, tag="ph")  # one bank
                    jsl = slice(j * P, (j + 1) * P)
                    for cc in range(NCC):
                        cs = slice(cc * P, (cc + 1) * P)
                        nc.tensor.matmul(ph[:, cc, :], lhsT=E[:, j, cs],
                                         rhs=m1_sb[:, j, :],
                                         start=True, stop=False)
                        nc.tensor.matmul(ph[:, cc, :], lhsT=s_bf[0:1, cs],
                                         rhs=rrow_sb[0:1, jsl],
                                         start=False, stop=True)
                    for cc in range(NCC):
                        nc.vector.tensor_copy(hTpre[:, cc, jsl],
                                              ph[:, cc, :])
                    if j < NLOC - 1:
                        ps_cs = pss.tile([1, C], F32, tag="cs")
                        nc.tensor.matmul(ps_cs[:], lhsT=ones_sb[:],
                                         rhs=E[:, j, :],
                                         start=True, stop=True)
                        s_new = ssp.tile([1, C], F32, tag="sf")
                        nc.vector.tensor_add(s_new[:], s_f32[:], ps_cs[:])
                        s_bf = ssp.tile([1, C], BF16, tag="sb")
                        nc.vector.tensor_copy(s_bf[:], s_new[:])
                        s_f32 = s_new

            # ---------------- Phase C: MLP (local half) ----------------
            wmats = stack_bc.enter_context(tc.tile_pool(name="wmats", bufs=1))
            wfc_sb = wmats.tile([P, NCC, H], BF16, tag="wfc")
            nc.sync.dma_start(out=wfc_sb[:], in_=w_fc[:])
            wproj_sb = wmats.tile([P, NHC, C], BF16, tag="wproj")
            nc.scalar.dma_start(out=wproj_sb[:], in_=w_proj[:])
            # preload head weight groups behind wfc/wproj on the sync queue:
            # FIFO order gives the MLP weights HBM priority, then these 6MB
            # stream in well before the head starts.
            whs_next = [load_group(VGROUPS[0]), load_group(VGROUPS[1]),
                        load_group(VGROUPS[2])]
            with tc.tile_pool(name="ap_", bufs=NHC) as ap_, \
                 tc.tile_pool(name="ctmp", bufs=3) as ctmp, \
                 tc.tile_pool(name="psfc", bufs=2, space="PSUM") as psfc, \
                 tc.tile_pool(name="pspj", bufs=2, space="PSUM") as pspj:
                for gidx in range(TLOC // TG):
                    gsl = slice(gidx * TG, (gidx + 1) * TG)
                    a_tiles = []
                    for hc in range(NHC):
                        pfc = psfc.tile([P, TG], F32, tag="fc")
                        for c in range(NCC):
                            nc.tensor.matmul(
                                pfc[:], lhsT=wfc_sb[:, c, hc * P:(hc + 1) * P],
                                rhs=hTpre[:, c, gsl],
                                start=(c == 0), stop=(c == NCC - 1))
                        a = ap_.tile([P, TG], BF16, tag="a")
                        nc.scalar.activation(a[:], pfc[:],
                                             mybir.ActivationFunctionType.Tanh,
                                             bias=bfc_sb[:, hc:hc + 1])
                        a_tiles.append(a)
                    for cc in range(NCC):
                        pproj = pspj.tile([P, TG], F32, tag="proj")
                        for hc in range(NHC):
                            nc.tensor.matmul(
                                pproj[:],
                                lhsT=wproj_sb[:, hc, cc * P:(cc + 1) * P],
                                rhs=a_tiles[hc][:],
                                start=(hc == 0), stop=(hc == NHC - 1))
                        tmpc = ctmp.tile([P, TG], F32, tag="tmpc")
                        nc.scalar.activation(tmpc[:], pproj[:],
                                             mybir.ActivationFunctionType.Identity,
                                             bias=bproj_sb[:, cc:cc + 1])
                        nc.vector.tensor_add(hF[:, cc, gsl], tmpc[:],
                                             hTpre[:, cc, gsl])
                    # pair AllReduce of this 512-token group (bf16),
                    # overlapped with the own-half head matmuls.
                    nc.sync.dma_start(out=cc_in[gidx][:], in_=hF[:, :, gsl])
                    nc.gpsimd.collective_compute(
                        "AllReduce",
                        mybir.AluOpType.add,
                        replica_groups=[[0, 1], [2, 3], [4, 5], [6, 7]],
                        ins=[cc_in[gidx][:].opt()],
                        outs=[cc_red[gidx][:].opt()],
                    )

            # ---------------- Phase D: head ----------------
            stack_bc.close()  # free wfc/wproj + hTpre SBUF for the head
            with tc.tile_pool(name="smp", bufs=2) as smp, \
                 tc.tile_pool(name="stp", bufs=4) as stp, \
                 tc.tile_pool(name="pso", bufs=8, space="PSUM") as pso:
                gseq = 0
                for pss_ in range(2):
                    for grp_i, group in enumerate(VGROUPS):
                        if pss_ == 0 and grp_i == 6:
                            # peer half = allreduce sum - own half; done
                            # mid-pass-0 so pass 1 never waits on it
                            for gidx in range(2):
                                gsl = slice(gidx * TG, (gidx + 1) * TG)
                                psl = slice(TLOC + gidx * TG,
                                            TLOC + (gidx + 1) * TG)
                                sm = smp.tile([P, NCC, TG], BF16, tag="sm")
                                nc.sync.dma_start(out=sm[:],
                                                  in_=cc_red[gidx][:])
                                for cc in range(NCC):
                                    nc.vector.tensor_tensor(
                                        out=hF[:, cc, psl],
                                        in0=sm[:, cc, :],
                                        in1=hF[:, cc, gsl],
                                        op=mybir.AluOpType.subtract)
                        whs = whs_next[0]
                        whs_next = whs_next[1:]
                        gseq += 1
                        if gseq + 2 < 2 * len(VGROUPS):
                            nxt = VGROUPS[(gseq + 2) % len(VGROUPS)]
                            whs_next.append(load_group(nxt))
                        v0 = group[0][0]
                        gw = sum(w for _, w in group)
                        for j in range(NLOC):
                            tok = pss_ * TLOC + j * P
                            lrow = (pss_ * NLOC + j) * P
                            psums = []
                            for _vi in range(len(group)):
                                po = pso.tile([P, VT], F32, tag="po")
                                psums.append(po)
                            for c in range(NCC):
                                for vi, (_, w) in enumerate(group):
                                    nc.tensor.matmul(
                                        psums[vi][:, :w],
                                        lhsT=hF[:, c, tok:tok + P],
                                        rhs=whs[vi][:, c, :w],
                                        start=(c == 0), stop=(c == NCC - 1))
                            st = stp.tile([P, 4 * VT], BF16, tag="st")
                            off = 0
                            for vi, (_, w) in enumerate(group):
                                dst = st[:, off:off + w]
                                off += w
                                if vi % 2:
                                    nc.scalar.activation(
                                        dst, psums[vi][:, :w],
                                        mybir.ActivationFunctionType.Copy)
                                else:
                                    nc.vector.tensor_copy(dst,
                                                          psums[vi][:, :w])
                            nc.sync.dma_start(
                                out=out[lrow:lrow + P, v0:v0 + gw],
                                in_=st[:, :gw])
    nc.compile()
    return nc


_NC = None


def _get_nc():
    global _NC
    if _NC is None:
        _NC = _build_nc()
    return _NC


def make_in_maps(x, wte, wpe, w_fc, b_fc, w_proj, b_proj, w_head, b_head):
    x = np.asarray(x).astype(np.int64)
    wte_f = np.asarray(wte, np.float32).astype(BF).astype(np.float32)
    wpe_f = np.asarray(wpe, np.float32).astype(BF).astype(np.float32)
    wfc_b = np.ascontiguousarray(
        np.asarray(w_fc, np.float32).astype(BF)
        .reshape(NCC, P, H).transpose(1, 0, 2))
    wproj_b = np.ascontiguousarray(
        np.asarray(w_proj, np.float32).astype(BF)
        .reshape(NHC, P, C).transpose(1, 0, 2))
    whead_b = np.asarray(w_head, np.float32).astype(BF)
    b_fc = np.asarray(b_fc, dtype=np.float32)
    b_proj = np.asarray(b_proj, dtype=np.float32)

    wh_halves = []
    for vh in range(2):
        lo = vh * VSPLIT
        hi = min(V, lo + VSPLIT)
        pad = np.zeros((C, VHALF_PAD), BF)
        pad[:, :hi - lo] = whead_b[:, lo:hi]
        wh_halves.append(pad)

    # per-half block permutation: own half's blocks first
    orders = [list(range(vh * NLOC, vh * NLOC + NLOC)) +
              list(range((1 - vh) * NLOC, (1 - vh) * NLOC + NLOC))
              for vh in range(2)]

    # host-side embedding gather: emb[b] = wte[x[b]] + wpe, bf16,
    # laid out [token-in-block, block, C] in each core's block order
    embs = []
    for b in range(B):
        e = (wte_f[x[b]] + wpe_f).astype(BF)          # (T, C)
        embs.append(e.reshape(NBLK, P, C))
    emb_cores = []
    for core in range(8):
        b, vh = core // 2, core % 2
        e = embs[b][orders[vh]]                        # (NBLK, P, C)
        emb_cores.append(np.ascontiguousarray(e.transpose(1, 0, 2)))

    # per-half M1 (I + triu*recip per block) and recip row, both bf16
    m1s, rrows = [], []
    for vh in range(2):
        m1 = np.zeros((P, NLOC, P), np.float32)
        rr = np.zeros((1, TLOC), np.float32)
        for j in range(NLOC):
            gblk = vh * NLOC + j
            tglob = gblk * P + np.arange(P) + 1  # 1-indexed positions
            recip = (1.0 / tglob).astype(np.float32)
            m1[:, j, :] = (np.triu(np.ones((P, P), np.float32))
                           * recip[None, :] + np.eye(P, dtype=np.float32))
            rr[0, j * P:(j + 1) * P] = recip
        m1s.append(m1.astype(BF))
        rrows.append(rr.astype(BF))

    b_fc2d = np.ascontiguousarray(b_fc.reshape(NHC, P).T)
    b_proj2d = np.ascontiguousarray(b_proj.reshape(NCC, P).T)
    ones_col = np.ones((P, 1), BF)

    in_maps = []
    for core in range(8):
        vh = core % 2
        in_maps.append({
            "emb": emb_cores[core],
            "w_fc": wfc_b,
            "w_proj": wproj_b,
            "w_head": wh_halves[vh],
            "b_fc2d": b_fc2d,
            "b_proj2d": b_proj2d,
            "m1": m1s[vh],
            "rrow": rrows[vh],
            "mask": np.full((1, 1), float(vh), np.float32),
            "ones_col": ones_col,
        })
    return in_maps


def kernel(x, wte, wpe, w_fc, b_fc, w_proj, b_proj, w_head, b_head):
    b_head = np.asarray(b_head, dtype=np.float32)
    in_maps = make_in_maps(x, wte, wpe, w_fc, b_fc, w_proj, b_proj,
                           w_head, b_head)
    nc = _get_nc()
    res = run_bass_kernel_spmd(nc, in_maps, core_ids=list(range(8)))

    logits = np.empty((B, T, V), np.float32)
    for core in range(8):
        b = core // 2
        vh = core % 2
        lo = vh * VSPLIT
        hi = min(V, lo + VSPLIT)
        co = np.asarray(res.results[core]["out"])
        co = co.view(np.uint16).astype(np.uint32) << 16
        co = co.view(np.float32)[:, :hi - lo]
        # rows are in local block order: own half first
        logits[b, vh * TLOC:vh * TLOC + TLOC, lo:hi] = co[:TLOC]
        logits[b, (1 - vh) * TLOC:(1 - vh) * TLOC + TLOC, lo:hi] = co[TLOC:]
    if b_head.any():
        logits += b_head[None, None, :]
    return logits


# revision 12
# speedup vs baseline: 1.0004x; 1.0004x over previous
"""Trainium2 Bass kernel for the BoW language model head problem.

Model (per reference):
    emb = wte[x] + wpe            (B,T,C)
    h   = emb + cumsum(emb)/[1..T]
    h   = h + tanh(h@w_fc+b_fc)@w_proj + b_proj
    out = h @ w_head + b_head     (B,T,V)

Shapes: B=4, T=2048, V=50257, C=512, H=2048.

Sharding (8 cores): core i computes batch i//2 and vocab half i%2.
Pre-head compute is split across the pair by tokens (each core does its
own 1024-token half); a bf16 AllReduce per 512-token group plus a
subtract reconstructs the peer half, overlapped with the own-half head
matmuls.  Data is bf16 end-to-end with fp32 PSUM accumulation; the
embedding gather runs on the host, and the causal-BoW cumsum folds 1/t
into a per-block matrix M1 = I + triu*recip so its matmul lands h
directly in C-major layout (no PE transposes).  The head streams w_head
tiles (moving, N=512) against stationary h blocks; output is written
bf16 and up-converted on the host.
"""

from contextlib import ExitStack

import numpy as np
import ml_dtypes

import concourse.bacc as bacc
import concourse.bass as bass
import concourse.mybir as mybir
import concourse.tile as tile
from concourse.bass_utils import run_bass_kernel_spmd

P = 128
B, T, V, C, H = 4, 2048, 50257, 512, 2048
NBLK = T // P          # 16 token blocks
NLOC = NBLK // 2       # 8 local token blocks per core (pair-split pre-head)
TLOC = NLOC * P        # 1024 local tokens
NCC = C // P           # 4 C chunks
NHC = H // P           # 16 H chunks
TG = 512               # token group width (MLP moving dim)
VT = 512               # vocab tile width
VSPLIT = (V + 1) // 2  # 25129: half0 = [:VSPLIT], half1 = [VSPLIT:]
VHALF_PAD = 25216      # VSPLIT padded to a multiple of 128 (49*512 + 128)
# vocab tile widths and groups (~2048 elements per group -> 4 PSUM banks
# per j-block, ping-pong across j)
_TILES = [(i * VT, VT) for i in range(VHALF_PAD // VT)]
if VHALF_PAD % VT:
    _TILES.append((VHALF_PAD - VHALF_PAD % VT, VHALF_PAD % VT))
VGROUPS = [_TILES[i:i + 4] for i in range(0, len(_TILES), 4)]

F32 = mybir.dt.float32
BF16 = mybir.dt.bfloat16

BF = ml_dtypes.bfloat16


def _build_nc():
    nc = bacc.Bacc(None, target_bir_lowering=False, debug=True,
                   num_swdge_queues=4, num_devices=8)

    emb = nc.dram_tensor("emb", [P, NBLK, C], BF16, kind="ExternalInput")
    w_fc = nc.dram_tensor("w_fc", [P, NCC, H], BF16, kind="ExternalInput")
    w_proj = nc.dram_tensor("w_proj", [P, NHC, C], BF16, kind="ExternalInput")
    w_head = nc.dram_tensor("w_head", [C, VHALF_PAD], BF16,
                            kind="ExternalInput")
    b_fc2d = nc.dram_tensor("b_fc2d", [P, NHC], F32, kind="ExternalInput")
    b_proj2d = nc.dram_tensor("b_proj2d", [P, NCC], F32, kind="ExternalInput")
    m1 = nc.dram_tensor("m1", [P, NLOC, P], BF16, kind="ExternalInput")
    rrow = nc.dram_tensor("rrow", [1, TLOC], BF16, kind="ExternalInput")
    mask = nc.dram_tensor("mask", [1, 1], F32, kind="ExternalInput")
    ones_col = nc.dram_tensor("ones_col", [P, 1], BF16, kind="ExternalInput")
    out = nc.dram_tensor("out", [T, VHALF_PAD], BF16, kind="ExternalOutput")

    with tile.TileContext(nc) as tc:
        with tc.tile_pool(name="consts", bufs=1) as consts, \
             tc.tile_pool(name="hfp", bufs=1) as hfp, \
             tc.tile_pool(name="whp", bufs=16) as whp, \
             tc.tile_pool(name="ccdr", bufs=1, space="DRAM") as ccdr:
            ones_sb = consts.tile([P, 1], BF16, tag="ones")
            nc.sync.dma_start(out=ones_sb[:], in_=ones_col[:])

            # PE warm-up: ~32 junk matmuls bridge the HAM activity window so
            # the real phases start at 2.4GHz instead of the cold 1.2GHz.
            with tc.tile_pool(name="pwarm", bufs=1, space="PSUM") as pwarm, \
                 tc.tile_pool(name="wscp", bufs=1) as wscp:
                wsc = wscp.tile([P, 512], BF16, tag="wsc")
                nc.vector.memset(wsc[:], 0.0)
                wps = pwarm.tile([P, 512], F32, tag="wps")
                for _ in range(32):
                    nc.tensor.matmul(wps[:], lhsT=wsc[:, 0:P], rhs=wsc[:],
                                     start=True, stop=True)

            # hF holds post-MLP h (C-major, bf16): own tokens 0:1024,
            # peer tokens 1024:2048 (reconstructed after the AllReduce).
            hF = hfp.tile([P, NCC, T], BF16, tag="hF")
            cc_in = [ccdr.tile([P, NCC, TG], BF16, tag=f"cci{g}",
                               name=f"cc_in{g}")
                     for g in range(2)]
            cc_red = [ccdr.tile([P, NCC, TG], BF16, tag=f"ccr{g}",
                                name=f"cc_red{g}")
                      for g in range(2)]

            wh_view = w_head.rearrange("(c p) v -> p c v", p=P)

            def load_group(group):
                whs = []
                for v0, w in group:
                    wh = whp.tile([P, NCC, VT], BF16, tag="wh")
                    nc.sync.dma_start(out=wh[:, :, :w],
                                        in_=wh_view[:, :, v0:v0 + w])
                    whs.append(wh)
                return whs

            stack_bc = ExitStack()
            htp = stack_bc.enter_context(tc.tile_pool(name="htp", bufs=1))
            hTpre = htp.tile([P, NCC, TLOC], BF16, tag="hTpre")

            # ---------------- Phase B: causal BoW ----------
            with tc.tile_pool(name="ebuf", bufs=1) as ebuf, \
                 tc.tile_pool(name="ssp", bufs=3) as ssp, \
                 tc.tile_pool(name="pss", bufs=1, space="PSUM") as pss, \
                 tc.tile_pool(name="psw", bufs=1, space="PSUM") as psw, \
                 tc.tile_pool(name="psh", bufs=2, space="PSUM") as psh:
                E = ebuf.tile([P, NBLK, C], BF16, tag="E")
                # spread the critical early loads across the three DMA
                # paths: peer-E on sync, own-E on scalar, consts on gpsimd
                nc.sync.dma_start(out=E[:, NLOC:, :], in_=emb[:, NLOC:, :])
                nc.scalar.dma_start(out=E[:, :NLOC, :], in_=emb[:, :NLOC, :])
                m1_sb = consts.tile([P, NLOC, P], BF16, tag="m1")
                nc.gpsimd.dma_start(out=m1_sb[:], in_=m1[:])
                rrow_sb = consts.tile([1, TLOC], BF16, tag="rrow")
                nc.gpsimd.dma_start(out=rrow_sb[:], in_=rrow[:])
                mask_sb = consts.tile([1, 1], F32, tag="mask")
                nc.gpsimd.dma_start(out=mask_sb[:], in_=mask[:])
                bfc_sb = consts.tile([P, NHC], F32, tag="bfc")
                nc.gpsimd.dma_start(out=bfc_sb[:], in_=b_fc2d[:])
                bproj_sb = consts.tile([P, NCC], F32, tag="bproj")
                nc.gpsimd.dma_start(out=bproj_sb[:], in_=b_proj2d[:])

                # prefix base: peer colsum (masked), then block colsums
                # in two packed PSUM waves + one DVE prefix pass -- a single
                # PE->DVE->PE round instead of one per block.
                ps_base = pss.tile([1, C], F32, tag="pbase")
                for j in range(NLOC, NBLK):
                    nc.tensor.matmul(ps_base[:], lhsT=ones_sb[:],
                                     rhs=E[:, j, :],
                                     start=(j == NLOC), stop=(j == NBLK - 1))
                waves = [list(range(0, 4)), list(range(4, NLOC - 1))]
                cs_tiles = []
                for wv in waves:
                    csw = psw.tile([1, 4, C], F32, tag="csw")
                    cs_tiles.append(csw)
                    for k, j in enumerate(wv):
                        nc.tensor.matmul(csw[0:1, k, :], lhsT=ones_sb[:],
                                         rhs=E[:, j, :],
                                         start=True, stop=True)
                # DVE prefix: s_all[j] = mask*peer_total + colsums 0..j-1
                s_all = ssp.tile([1, NLOC, C], BF16, tag="sall")
                s_run = ssp.tile([1, C], F32, tag="sf")
                nc.vector.tensor_scalar_mul(s_run[:], ps_base[:],
                                            mask_sb[:, :1])
                nc.vector.tensor_copy(s_all[0:1, 0, :], s_run[:])
                for j in range(1, NLOC):
                    wv, k = (0, j - 1) if j <= 4 else (1, j - 5)
                    s_new = ssp.tile([1, C], F32, tag="sf")
                    nc.vector.tensor_add(s_new[:], s_run[:],
                                         cs_tiles[wv][0:1, k, :])
                    nc.vector.tensor_copy(s_all[0:1, j, :], s_new[:])
                    s_run = s_new

                for j in range(NLOC):
                    ph = psh.tile([P, NCC, P], F32, tag="ph")  # one bank
                    jsl = slice(j * P, (j + 1) * P)
                    for cc in range(NCC):
                        cs = slice(cc * P, (cc + 1) * P)
                        nc.tensor.matmul(ph[:, cc, :], lhsT=E[:, j, cs],
                                         rhs=m1_sb[:, j, :],
                                         start=True, stop=False)
                        nc.tensor.matmul(ph[:, cc, :],
                                         lhsT=s_all[0:1, j, cs],
                                         rhs=rrow_sb[0:1, jsl],
                                         start=False, stop=True)
                    for cc in range(NCC):
                        nc.vector.tensor_copy(hTpre[:, cc, jsl],
                                              ph[:, cc, :])

            # ---------------- Phase C: MLP (local half) ----------------
            wmats = stack_bc.enter_context(tc.tile_pool(name="wmats", bufs=1))
            wfc_sb = wmats.tile([P, NCC, H], BF16, tag="wfc")
            nc.sync.dma_start(out=wfc_sb[:], in_=w_fc[:])
            wproj_sb = wmats.tile([P, NHC, C], BF16, tag="wproj")
            nc.scalar.dma_start(out=wproj_sb[:], in_=w_proj[:])
            # preload head weight groups behind wfc/wproj on the sync queue:
            # FIFO order gives the MLP weights HBM priority, then these 6MB
            # stream in well before the head starts.
            whs_next = [load_group(VGROUPS[0]), load_group(VGROUPS[1]),
                        load_group(VGROUPS[2])]
            with tc.tile_pool(name="ap_", bufs=NHC) as ap_, \
                 tc.tile_pool(name="ctmp", bufs=3) as ctmp, \
                 tc.tile_pool(name="psfc", bufs=2, space="PSUM") as psfc, \
                 tc.tile_pool(name="pspj", bufs=2, space="PSUM") as pspj:
                for gidx in range(TLOC // TG):
                    gsl = slice(gidx * TG, (gidx + 1) * TG)
                    a_tiles = []
                    for hc in range(NHC):
                        pfc = psfc.tile([P, TG], F32, tag="fc")
                        for c in range(NCC):
                            nc.tensor.matmul(
                                pfc[:], lhsT=wfc_sb[:, c, hc * P:(hc + 1) * P],
                                rhs=hTpre[:, c, gsl],
                                start=(c == 0), stop=(c == NCC - 1))
                        a = ap_.tile([P, TG], BF16, tag="a")
                        nc.scalar.activation(a[:], pfc[:],
                                             mybir.ActivationFunctionType.Tanh,
                                             bias=bfc_sb[:, hc:hc + 1])
                        a_tiles.append(a)
                    for cc in range(NCC):
                        pproj = pspj.tile([P, TG], F32, tag="proj")
                        for hc in range(NHC):
                            nc.tensor.matmul(
                                pproj[:],
                                lhsT=wproj_sb[:, hc, cc * P:(cc + 1) * P],
                                rhs=a_tiles[hc][:],
                                start=(hc == 0), stop=(hc == NHC - 1))
                        tmpc = ctmp.tile([P, TG], F32, tag="tmpc")
                        nc.scalar.activation(tmpc[:], pproj[:],
                                             mybir.ActivationFunctionType.Identity,
                                             bias=bproj_sb[:, cc:cc + 1])
                        nc.vector.tensor_add(hF[:, cc, gsl], tmpc[:],
                                             hTpre[:, cc, gsl])
                    # pair AllReduce of this 512-token group (bf16),
                    # overlapped with the own-half head matmuls.
                    nc.sync.dma_start(out=cc_in[gidx][:], in_=hF[:, :, gsl])
                    nc.gpsimd.collective_compute(
                        "AllReduce",
                        mybir.AluOpType.add,
                        replica_groups=[[0, 1], [2, 3], [4, 5], [6, 7]],
                        ins=[cc_in[gidx][:].opt()],
                        outs=[cc_red[gidx][:].opt()],
                    )

            # ---------------- Phase D: head ----------------
            stack_bc.close()  # free wfc/wproj + hTpre SBUF for the head
            with tc.tile_pool(name="smp", bufs=2) as smp, \
                 tc.tile_pool(name="stp", bufs=4) as stp, \
                 tc.tile_pool(name="pso", bufs=8, space="PSUM") as pso:
                gseq = 0
                for pss_ in range(2):
                    for grp_i, group in enumerate(VGROUPS):
                        if pss_ == 0 and grp_i == 6:
                            # peer half = allreduce sum - own half; done
                            # mid-pass-0 so pass 1 never waits on it
                            for gidx in range(2):
                                gsl = slice(gidx * TG, (gidx + 1) * TG)
                                psl = slice(TLOC + gidx * TG,
                                            TLOC + (gidx + 1) * TG)
                                sm = smp.tile([P, NCC, TG], BF16, tag="sm")
                                nc.sync.dma_start(out=sm[:],
                                                  in_=cc_red[gidx][:])
                                for cc in range(NCC):
                                    nc.vector.tensor_tensor(
                                        out=hF[:, cc, psl],
                                        in0=sm[:, cc, :],
                                        in1=hF[:, cc, gsl],
                                        op=mybir.AluOpType.subtract)
                        whs = whs_next[0]
                        whs_next = whs_next[1:]
                        gseq += 1
                        if gseq + 2 < 2 * len(VGROUPS):
                            nxt = VGROUPS[(gseq + 2) % len(VGROUPS)]
                            whs_next.append(load_group(nxt))
                        v0 = group[0][0]
                        gw = sum(w for _, w in group)
                        for j in range(NLOC):
                            tok = pss_ * TLOC + j * P
                            lrow = (pss_ * NLOC + j) * P
                            psums = []
                            for _vi in range(len(group)):
                                po = pso.tile([P, VT], F32, tag="po")
                                psums.append(po)
                            for c in range(NCC):
                                for vi, (_, w) in enumerate(group):
                                    nc.tensor.matmul(
                                        psums[vi][:, :w],
                                        lhsT=hF[:, c, tok:tok + P],
                                        rhs=whs[vi][:, c, :w],
                                        start=(c == 0), stop=(c == NCC - 1))
                            st = stp.tile([P, 4 * VT], BF16, tag="st")
                            off = 0
                            for vi, (_, w) in enumerate(group):
                                dst = st[:, off:off + w]
                                off += w
                                if vi % 2:
                                    nc.scalar.activation(
                                        dst, psums[vi][:, :w],
                                        mybir.ActivationFunctionType.Copy)
                                else:
                                    nc.vector.tensor_copy(dst,
                                                          psums[vi][:, :w])
                            nc.sync.dma_start(
                                out=out[lrow:lrow + P, v0:v0 + gw],
                                in_=st[:, :gw])
    nc.compile()
    return nc


_NC = None


def _get_nc():
    global _NC
    if _NC is None:
        _NC = _build_nc()
    return _NC


def make_in_maps(x, wte, wpe, w_fc, b_fc, w_proj, b_proj, w_head, b_head):
    x = np.asarray(x).astype(np.int64)
    wte_f = np.asarray(wte, np.float32).astype(BF).astype(np.float32)
    wpe_f = np.asarray(wpe, np.float32).astype(BF).astype(np.float32)
    wfc_b = np.ascontiguousarray(
        np.asarray(w_fc, np.float32).astype(BF)
        .reshape(NCC, P, H).transpose(1, 0, 2))
    wproj_b = np.ascontiguousarray(
        np.asarray(w_proj, np.float32).astype(BF)
        .reshape(NHC, P, C).transpose(1, 0, 2))
    whead_b = np.asarray(w_head, np.float32).astype(BF)
    b_fc = np.asarray(b_fc, dtype=np.float32)
    b_proj = np.asarray(b_proj, dtype=np.float32)

    wh_halves = []
    for vh in range(2):
        lo = vh * VSPLIT
        hi = min(V, lo + VSPLIT)
        pad = np.zeros((C, VHALF_PAD), BF)
        pad[:, :hi - lo] = whead_b[:, lo:hi]
        wh_halves.append(pad)

    # per-half block permutation: own half's blocks first
    orders = [list(range(vh * NLOC, vh * NLOC + NLOC)) +
              list(range((1 - vh) * NLOC, (1 - vh) * NLOC + NLOC))
              for vh in range(2)]

    # host-side embedding gather: emb[b] = wte[x[b]] + wpe, bf16,
    # laid out [token-in-block, block, C] in each core's block order
    embs = []
    for b in range(B):
        e = (wte_f[x[b]] + wpe_f).astype(BF)          # (T, C)
        embs.append(e.reshape(NBLK, P, C))
    emb_cores = []
    for core in range(8):
        b, vh = core // 2, core % 2
        e = embs[b][orders[vh]]                        # (NBLK, P, C)
        emb_cores.append(np.ascontiguousarray(e.transpose(1, 0, 2)))

    # per-half M1 (I + triu*recip per block) and recip row, both bf16
    m1s, rrows = [], []
    for vh in range(2):
        m1 = np.zeros((P, NLOC, P), np.float32)
        rr = np.zeros((1, TLOC), np.float32)
        for j in range(NLOC):
            gblk = vh * NLOC + j
            tglob = gblk * P + np.arange(P) + 1  # 1-indexed positions
            recip = (1.0 / tglob).astype(np.float32)
            m1[:, j, :] = (np.triu(np.ones((P, P), np.float32))
                           * recip[None, :] + np.eye(P, dtype=np.float32))
            rr[0, j * P:(j + 1) * P] = recip
        m1s.append(m1.astype(BF))
        rrows.append(rr.astype(BF))

    b_fc2d = np.ascontiguousarray(b_fc.reshape(NHC, P).T)
    b_proj2d = np.ascontiguousarray(b_proj.reshape(NCC, P).T)
    ones_col = np.ones((P, 1), BF)

    in_maps = []
    for core in range(8):
        vh = core % 2
        in_maps.append({
            "emb": emb_cores[core],
            "w_fc": wfc_b,
            "w_proj": wproj_b,
            "w_head": wh_halves[vh],
            "b_fc2d": b_fc2d,
            "b_proj2d": b_proj2d,
            "m1": m1s[vh],
            "rrow": rrows[vh],
            "mask": np.full((1, 1), float(vh), np.float32),
            "ones_col": ones_col,
        })
    return in_maps


def kernel(x, wte, wpe, w_fc, b_fc, w_proj, b_proj, w_head, b_head):
    b_head = np.asarray(b_head, dtype=np.float32)
    in_maps = make_in_maps(x, wte, wpe, w_fc, b_fc, w_proj, b_proj,
                           w_head, b_head)
    nc = _get_nc()
    res = run_bass_kernel_spmd(nc, in_maps, core_ids=list(range(8)))

    logits = np.empty((B, T, V), np.float32)
    for core in range(8):
        b = core // 2
        vh = core % 2
        lo = vh * VSPLIT
        hi = min(V, lo + VSPLIT)
        co = np.asarray(res.results[core]["out"])
        co = co.view(np.uint16).astype(np.uint32) << 16
        co = co.view(np.float32)[:, :hi - lo]
        # rows are in local block order: own half first
        logits[b, vh * TLOC:vh * TLOC + TLOC, lo:hi] = co[:TLOC]
        logits[b, (1 - vh) * TLOC:(1 - vh) * TLOC + TLOC, lo:hi] = co[TLOC:]
    if b_head.any():
        logits += b_head[None, None, :]
    return logits


# revision 13
# speedup vs baseline: 1.0147x; 1.0143x over previous
"""Trainium2 Bass kernel for the BoW language model head problem.

Model (per reference):
    emb = wte[x] + wpe            (B,T,C)
    h   = emb + cumsum(emb)/[1..T]
    h   = h + tanh(h@w_fc+b_fc)@w_proj + b_proj
    out = h @ w_head + b_head     (B,T,V)

Shapes: B=4, T=2048, V=50257, C=512, H=2048.

Sharding (8 cores): core i computes batch i//2 and vocab half i%2.
Pre-head compute is split across the pair by tokens (each core does its
own 1024-token half); a bf16 AllReduce per 512-token group plus a
subtract reconstructs the peer half, overlapped with the own-half head
matmuls.  Data is bf16 end-to-end with fp32 PSUM accumulation; the
embedding gather runs on the host, and the causal-BoW cumsum folds 1/t
into a per-block matrix M1 = I + triu*recip so its matmul lands h
directly in C-major layout (no PE transposes).  The head streams w_head
tiles (moving, N=512) against stationary h blocks; output is written
bf16 and up-converted on the host.
"""

from contextlib import ExitStack

import numpy as np
import ml_dtypes

import concourse.bacc as bacc
import concourse.bass as bass
import concourse.mybir as mybir
import concourse.tile as tile
from concourse.bass_utils import run_bass_kernel_spmd

P = 128
B, T, V, C, H = 4, 2048, 50257, 512, 2048
NBLK = T // P          # 16 token blocks
NLOC = NBLK // 2       # 8 local token blocks per core (pair-split pre-head)
TLOC = NLOC * P        # 1024 local tokens
NCC = C // P           # 4 C chunks
NHC = H // P           # 16 H chunks
TG = 512               # token group width (MLP moving dim)
VT = 512               # vocab tile width
VSPLIT = (V + 1) // 2  # 25129: half0 = [:VSPLIT], half1 = [VSPLIT:]
VHALF_PAD = 25216      # VSPLIT padded to a multiple of 128 (49*512 + 128)
# vocab tile widths and groups (~2048 elements per group -> 4 PSUM banks
# per j-block, ping-pong across j)
_TILES = [(i * VT, VT) for i in range(VHALF_PAD // VT)]
if VHALF_PAD % VT:
    _TILES.append((VHALF_PAD - VHALF_PAD % VT, VHALF_PAD % VT))
VGROUPS = [_TILES[i:i + 4] for i in range(0, len(_TILES), 4)]

F32 = mybir.dt.float32
BF16 = mybir.dt.bfloat16

BF = ml_dtypes.bfloat16


def _build_nc():
    nc = bacc.Bacc(None, target_bir_lowering=False, debug=True,
                   num_swdge_queues=4, num_devices=8)

    emb = nc.dram_tensor("emb", [P, NBLK, C], BF16, kind="ExternalInput")
    w_fc = nc.dram_tensor("w_fc", [P, NCC, H], BF16, kind="ExternalInput")
    w_proj = nc.dram_tensor("w_proj", [P, NHC, C], BF16, kind="ExternalInput")
    w_head = nc.dram_tensor("w_head", [C, VHALF_PAD], BF16,
                            kind="ExternalInput")
    b_fc2d = nc.dram_tensor("b_fc2d", [P, NHC], F32, kind="ExternalInput")
    b_proj2d = nc.dram_tensor("b_proj2d", [P, NCC], F32, kind="ExternalInput")
    m1 = nc.dram_tensor("m1", [P, NLOC, P], BF16, kind="ExternalInput")
    rrow = nc.dram_tensor("rrow", [1, TLOC], BF16, kind="ExternalInput")
    mask = nc.dram_tensor("mask", [1, 1], F32, kind="ExternalInput")
    ones_col = nc.dram_tensor("ones_col", [P, 1], BF16, kind="ExternalInput")
    out = nc.dram_tensor("out", [T, VHALF_PAD], BF16, kind="ExternalOutput")

    with tile.TileContext(nc) as tc:
        with tc.tile_pool(name="consts", bufs=1) as consts, \
             tc.tile_pool(name="hfp", bufs=1) as hfp, \
             tc.tile_pool(name="whp", bufs=16) as whp, \
             tc.tile_pool(name="ccdr", bufs=1, space="DRAM") as ccdr:
            ones_sb = consts.tile([P, 1], BF16, tag="ones")
            nc.sync.dma_start(out=ones_sb[:], in_=ones_col[:])

            # PE warm-up: ~32 junk matmuls bridge the HAM activity window so
            # the real phases start at 2.4GHz instead of the cold 1.2GHz.
            with tc.tile_pool(name="pwarm", bufs=1, space="PSUM") as pwarm, \
                 tc.tile_pool(name="wscp", bufs=1) as wscp:
                wsc = wscp.tile([P, 512], BF16, tag="wsc")
                nc.vector.memset(wsc[:], 0.0)
                wps = pwarm.tile([P, 512], F32, tag="wps")
                for _ in range(32):
                    nc.tensor.matmul(wps[:], lhsT=wsc[:, 0:P], rhs=wsc[:],
                                     start=True, stop=True)

            # hF holds post-MLP h (C-major, bf16): own tokens 0:1024,
            # peer tokens 1024:2048 (reconstructed after the AllReduce).
            hF = hfp.tile([P, NCC, T], BF16, tag="hF")
            cc_in = [ccdr.tile([P, NCC, TG], BF16, tag=f"cci{g}",
                               name=f"cc_in{g}")
                     for g in range(2)]
            cc_red = [ccdr.tile([P, NCC, TG], BF16, tag=f"ccr{g}",
                                name=f"cc_red{g}")
                      for g in range(2)]

            wh_view = w_head.rearrange("(c p) v -> p c v", p=P)

            def load_group(group):
                whs = []
                for v0, w in group:
                    wh = whp.tile([P, NCC, VT], BF16, tag="wh")
                    nc.sync.dma_start(out=wh[:, :, :w],
                                        in_=wh_view[:, :, v0:v0 + w])
                    whs.append(wh)
                return whs

            stack_bc = ExitStack()
            htp = stack_bc.enter_context(tc.tile_pool(name="htp", bufs=1))
            hTpre = htp.tile([P, NCC, TLOC], BF16, tag="hTpre")
            ebuf = stack_bc.enter_context(tc.tile_pool(name="ebuf", bufs=1))
            wmats = stack_bc.enter_context(tc.tile_pool(name="wmats", bufs=1))

            # spread the critical early loads across the three DMA paths:
            # peer-E + w_fc + head groups on sync, own-E + w_proj on scalar,
            # consts on gpsimd.  These pools all coexist, so no load waits
            # on SBUF reuse.
            E = ebuf.tile([P, NBLK, C], BF16, tag="E")
            nc.sync.dma_start(out=E[:, NLOC:, :], in_=emb[:, NLOC:, :])
            nc.scalar.dma_start(out=E[:, :NLOC, :], in_=emb[:, :NLOC, :])
            m1_sb = consts.tile([P, NLOC, P], BF16, tag="m1")
            nc.gpsimd.dma_start(out=m1_sb[:], in_=m1[:])
            rrow_sb = consts.tile([1, TLOC], BF16, tag="rrow")
            nc.gpsimd.dma_start(out=rrow_sb[:], in_=rrow[:])
            mask_sb = consts.tile([1, 1], F32, tag="mask")
            nc.gpsimd.dma_start(out=mask_sb[:], in_=mask[:])
            bfc_sb = consts.tile([P, NHC], F32, tag="bfc")
            nc.gpsimd.dma_start(out=bfc_sb[:], in_=b_fc2d[:])
            bproj_sb = consts.tile([P, NCC], F32, tag="bproj")
            nc.gpsimd.dma_start(out=bproj_sb[:], in_=b_proj2d[:])
            wfc_sb = wmats.tile([P, NCC, H], BF16, tag="wfc")
            nc.sync.dma_start(out=wfc_sb[:], in_=w_fc[:])
            wproj_sb = wmats.tile([P, NHC, C], BF16, tag="wproj")
            nc.scalar.dma_start(out=wproj_sb[:], in_=w_proj[:])
            whs_next = [load_group(VGROUPS[0]), load_group(VGROUPS[1]),
                        load_group(VGROUPS[2])]

            # ---------------- Phase B: causal BoW ----------
            with tc.tile_pool(name="ssp", bufs=3) as ssp, \
                 tc.tile_pool(name="pss", bufs=1, space="PSUM") as pss, \
                 tc.tile_pool(name="psw", bufs=1, space="PSUM") as psw, \
                 tc.tile_pool(name="psh", bufs=2, space="PSUM") as psh:
                # prefix base: peer colsum (masked), then block colsums
                # in two packed PSUM waves + one DVE prefix pass -- a single
                # PE->DVE->PE round instead of one per block.
                ps_base = pss.tile([1, C], F32, tag="pbase")
                for j in range(NLOC, NBLK):
                    nc.tensor.matmul(ps_base[:], lhsT=ones_sb[:],
                                     rhs=E[:, j, :],
                                     start=(j == NLOC), stop=(j == NBLK - 1))
                waves = [list(range(0, 4)), list(range(4, NLOC - 1))]
                cs_tiles = []
                for wv in waves:
                    csw = psw.tile([1, 4, C], F32, tag="csw")
                    cs_tiles.append(csw)
                    for k, j in enumerate(wv):
                        nc.tensor.matmul(csw[0:1, k, :], lhsT=ones_sb[:],
                                         rhs=E[:, j, :],
                                         start=True, stop=True)
                # DVE prefix: s_all[j] = mask*peer_total + colsums 0..j-1
                s_all = ssp.tile([1, NLOC, C], BF16, tag="sall")
                s_run = ssp.tile([1, C], F32, tag="sf")
                nc.vector.tensor_scalar_mul(s_run[:], ps_base[:],
                                            mask_sb[:, :1])
                nc.vector.tensor_copy(s_all[0:1, 0, :], s_run[:])
                for j in range(1, NLOC):
                    wv, k = (0, j - 1) if j <= 4 else (1, j - 5)
                    s_new = ssp.tile([1, C], F32, tag="sf")
                    nc.vector.tensor_add(s_new[:], s_run[:],
                                         cs_tiles[wv][0:1, k, :])
                    nc.vector.tensor_copy(s_all[0:1, j, :], s_new[:])
                    s_run = s_new

                for j in range(NLOC):
                    ph = psh.tile([P, NCC, P], F32, tag="ph")  # one bank
                    jsl = slice(j * P, (j + 1) * P)
                    for cc in range(NCC):
                        cs = slice(cc * P, (cc + 1) * P)
                        nc.tensor.matmul(ph[:, cc, :], lhsT=E[:, j, cs],
                                         rhs=m1_sb[:, j, :],
                                         start=True, stop=False)
                        nc.tensor.matmul(ph[:, cc, :],
                                         lhsT=s_all[0:1, j, cs],
                                         rhs=rrow_sb[0:1, jsl],
                                         start=False, stop=True)
                    for cc in range(NCC):
                        nc.vector.tensor_copy(hTpre[:, cc, jsl],
                                              ph[:, cc, :])

            # ---------------- Phase C: MLP (local half) ----------------
            with tc.tile_pool(name="ap_", bufs=NHC) as ap_, \
                 tc.tile_pool(name="ctmp", bufs=3) as ctmp, \
                 tc.tile_pool(name="psfc", bufs=2, space="PSUM") as psfc, \
                 tc.tile_pool(name="pspj", bufs=2, space="PSUM") as pspj:
                for gidx in range(TLOC // TG):
                    gsl = slice(gidx * TG, (gidx + 1) * TG)
                    a_tiles = []
                    for hc in range(NHC):
                        pfc = psfc.tile([P, TG], F32, tag="fc")
                        for c in range(NCC):
                            nc.tensor.matmul(
                                pfc[:], lhsT=wfc_sb[:, c, hc * P:(hc + 1) * P],
                                rhs=hTpre[:, c, gsl],
                                start=(c == 0), stop=(c == NCC - 1))
                        a = ap_.tile([P, TG], BF16, tag="a")
                        nc.scalar.activation(a[:], pfc[:],
                                             mybir.ActivationFunctionType.Tanh,
                                             bias=bfc_sb[:, hc:hc + 1])
                        a_tiles.append(a)
                    for cc in range(NCC):
                        pproj = pspj.tile([P, TG], F32, tag="proj")
                        for hc in range(NHC):
                            nc.tensor.matmul(
                                pproj[:],
                                lhsT=wproj_sb[:, hc, cc * P:(cc + 1) * P],
                                rhs=a_tiles[hc][:],
                                start=(hc == 0), stop=(hc == NHC - 1))
                        tmpc = ctmp.tile([P, TG], F32, tag="tmpc")
                        nc.scalar.activation(tmpc[:], pproj[:],
                                             mybir.ActivationFunctionType.Identity,
                                             bias=bproj_sb[:, cc:cc + 1])
                        nc.vector.tensor_add(hF[:, cc, gsl], tmpc[:],
                                             hTpre[:, cc, gsl])
                    # pair AllReduce of this 512-token group (bf16),
                    # overlapped with the own-half head matmuls.
                    nc.sync.dma_start(out=cc_in[gidx][:], in_=hF[:, :, gsl])
                    nc.gpsimd.collective_compute(
                        "AllReduce",
                        mybir.AluOpType.add,
                        replica_groups=[[0, 1], [2, 3], [4, 5], [6, 7]],
                        ins=[cc_in[gidx][:].opt()],
                        outs=[cc_red[gidx][:].opt()],
                    )

            # ---------------- Phase D: head ----------------
            stack_bc.close()  # free wfc/wproj + hTpre SBUF for the head
            with tc.tile_pool(name="smp", bufs=2) as smp, \
                 tc.tile_pool(name="stp", bufs=4) as stp, \
                 tc.tile_pool(name="pso", bufs=8, space="PSUM") as pso:
                gseq = 0
                for pss_ in range(2):
                    for grp_i, group in enumerate(VGROUPS):
                        if pss_ == 0 and grp_i == 6:
                            # peer half = allreduce sum - own half; done
                            # mid-pass-0 so pass 1 never waits on it
                            for gidx in range(2):
                                gsl = slice(gidx * TG, (gidx + 1) * TG)
                                psl = slice(TLOC + gidx * TG,
                                            TLOC + (gidx + 1) * TG)
                                sm = smp.tile([P, NCC, TG], BF16, tag="sm")
                                nc.sync.dma_start(out=sm[:],
                                                  in_=cc_red[gidx][:])
                                for cc in range(NCC):
                                    nc.vector.tensor_tensor(
                                        out=hF[:, cc, psl],
                                        in0=sm[:, cc, :],
                                        in1=hF[:, cc, gsl],
                                        op=mybir.AluOpType.subtract)
                        whs = whs_next[0]
                        whs_next = whs_next[1:]
                        gseq += 1
                        if gseq + 2 < 2 * len(VGROUPS):
                            nxt = VGROUPS[(gseq + 2) % len(VGROUPS)]
                            whs_next.append(load_group(nxt))
                        v0 = group[0][0]
                        gw = sum(w for _, w in group)
                        for j in range(NLOC):
                            tok = pss_ * TLOC + j * P
                            lrow = (pss_ * NLOC + j) * P
                            psums = []
                            for _vi in range(len(group)):
                                po = pso.tile([P, VT], F32, tag="po")
                                psums.append(po)
                            for c in range(NCC):
                                for vi, (_, w) in enumerate(group):
                                    nc.tensor.matmul(
                                        psums[vi][:, :w],
                                        lhsT=hF[:, c, tok:tok + P],
                                        rhs=whs[vi][:, c, :w],
                                        start=(c == 0), stop=(c == NCC - 1))
                            st = stp.tile([P, 4 * VT], BF16, tag="st")
                            off = 0
                            for vi, (_, w) in enumerate(group):
                                dst = st[:, off:off + w]
                                off += w
                                if vi % 2:
                                    nc.scalar.activation(
                                        dst, psums[vi][:, :w],
                                        mybir.ActivationFunctionType.Copy)
                                else:
                                    nc.vector.tensor_copy(dst,
                                                          psums[vi][:, :w])
                            nc.sync.dma_start(
                                out=out[lrow:lrow + P, v0:v0 + gw],
                                in_=st[:, :gw])
    nc.compile()
    return nc


_NC = None


def _get_nc():
    global _NC
    if _NC is None:
        _NC = _build_nc()
    return _NC


def make_in_maps(x, wte, wpe, w_fc, b_fc, w_proj, b_proj, w_head, b_head):
    x = np.asarray(x).astype(np.int64)
    wte_f = np.asarray(wte, np.float32).astype(BF).astype(np.float32)
    wpe_f = np.asarray(wpe, np.float32).astype(BF).astype(np.float32)
    wfc_b = np.ascontiguousarray(
        np.asarray(w_fc, np.float32).astype(BF)
        .reshape(NCC, P, H).transpose(1, 0, 2))
    wproj_b = np.ascontiguousarray(
        np.asarray(w_proj, np.float32).astype(BF)
        .reshape(NHC, P, C).transpose(1, 0, 2))
    whead_b = np.asarray(w_head, np.float32).astype(BF)
    b_fc = np.asarray(b_fc, dtype=np.float32)
    b_proj = np.asarray(b_proj, dtype=np.float32)

    wh_halves = []
    for vh in range(2):
        lo = vh * VSPLIT
        hi = min(V, lo + VSPLIT)
        pad = np.zeros((C, VHALF_PAD), BF)
        pad[:, :hi - lo] = whead_b[:, lo:hi]
        wh_halves.append(pad)

    # per-half block permutation: own half's blocks first
    orders = [list(range(vh * NLOC, vh * NLOC + NLOC)) +
              list(range((1 - vh) * NLOC, (1 - vh) * NLOC + NLOC))
              for vh in range(2)]

    # host-side embedding gather: emb[b] = wte[x[b]] + wpe, bf16,
    # laid out [token-in-block, block, C] in each core's block order
    embs = []
    for b in range(B):
        e = (wte_f[x[b]] + wpe_f).astype(BF)          # (T, C)
        embs.append(e.reshape(NBLK, P, C))
    emb_cores = []
    for core in range(8):
        b, vh = core // 2, core % 2
        e = embs[b][orders[vh]]                        # (NBLK, P, C)
        emb_cores.append(np.ascontiguousarray(e.transpose(1, 0, 2)))

    # per-half M1 (I + triu*recip per block) and recip row, both bf16
    m1s, rrows = [], []
    for vh in range(2):
        m1 = np.zeros((P, NLOC, P), np.float32)
        rr = np.zeros((1, TLOC), np.float32)
        for j in range(NLOC):
            gblk = vh * NLOC + j
            tglob = gblk * P + np.arange(P) + 1  # 1-indexed positions
            recip = (1.0 / tglob).astype(np.float32)
            m1[:, j, :] = (np.triu(np.ones((P, P), np.float32))
                           * recip[None, :] + np.eye(P, dtype=np.float32))
            rr[0, j * P:(j + 1) * P] = recip
        m1s.append(m1.astype(BF))
        rrows.append(rr.astype(BF))

    b_fc2d = np.ascontiguousarray(b_fc.reshape(NHC, P).T)
    b_proj2d = np.ascontiguousarray(b_proj.reshape(NCC, P).T)
    ones_col = np.ones((P, 1), BF)

    in_maps = []
    for core in range(8):
        vh = core % 2
        in_maps.append({
            "emb": emb_cores[core],
            "w_fc": wfc_b,
            "w_proj": wproj_b,
            "w_head": wh_halves[vh],
            "b_fc2d": b_fc2d,
            "b_proj2d": b_proj2d,
            "m1": m1s[vh],
            "rrow": rrows[vh],
            "mask": np.full((1, 1), float(vh), np.float32),
            "ones_col": ones_col,
        })
    return in_maps


def kernel(x, wte, wpe, w_fc, b_fc, w_proj, b_proj, w_head, b_head):
    b_head = np.asarray(b_head, dtype=np.float32)
    in_maps = make_in_maps(x, wte, wpe, w_fc, b_fc, w_proj, b_proj,
                           w_head, b_head)
    nc = _get_nc()
    res = run_bass_kernel_spmd(nc, in_maps, core_ids=list(range(8)))

    logits = np.empty((B, T, V), np.float32)
    for core in range(8):
        b = core // 2
        vh = core % 2
        lo = vh * VSPLIT
        hi = min(V, lo + VSPLIT)
        co = np.asarray(res.results[core]["out"])
        co = co.view(np.uint16).astype(np.uint32) << 16
        co = co.view(np.float32)[:, :hi - lo]
        # rows are in local block order: own half first
        logits[b, vh * TLOC:vh * TLOC + TLOC, lo:hi] = co[:TLOC]
        logits[b, (1 - vh) * TLOC:(1 - vh) * TLOC + TLOC, lo:hi] = co[TLOC:]
    if b_head.any():
        logits += b_head[None, None, :]
    return logits
